# revision 1
# baseline (speedup 1.0000x reference)
"""Trainium2 Bass kernel for BertSelfAttention with relative position embeddings.

Math (per batch b=1, S=384, H=768, NH=12, D=64):
  q/k/v = hs @ W{q,k,v}.T          (biases are zero in this problem -> skipped)
  a_c[h,q,k] = sum_d (q+u)[h,q,d] * k[h,k,d]
  b_d[h,q,k] = sum_F rel[q,k,F] * g[q,h,F],  g[q,h,F] = sum_d (q+v)[h,q,d]*Wr[h*64+d,F]
  out = softmax((a_c+b_d)/8 + mask) @ v

The g-reassociation avoids projecting the giant rel tensor through Wr
(64x FLOP reduction); the kernel is then memory-bound on streaming rel.

Sharding: query axis across 8 cores (48 q-rows each), no collectives.
Scores are built transposed [k, (h,q)] so the softmax sum runs on the PE
(ones-matmul over the partition dim) and ctx consumes probs directly.
"""

import numpy as np

S, H, NH, D = 384, 768, 12, 64
NCORES = 8
SQ = S // NCORES          # 48 q rows per core
KT = S // 128             # 3 k tiles
FC = H // 128             # 6 feature chunks
P = 128

_CACHED = {}


def build_kernel():
    import concourse.bass as bass
    import concourse.bacc as bacc
    import concourse.tile as tile
    from concourse import mybir
    from concourse.masks import make_identity

    f32 = mybir.dt.float32
    bf16 = mybir.dt.bfloat16
    EXP = mybir.ActivationFunctionType.Exp
    COPY = mybir.ActivationFunctionType.Copy

    nc = bacc.Bacc("TRN2", target_bir_lowering=False)

    hs = nc.dram_tensor("hs", [S, H], bf16, kind="ExternalInput")
    hs_loc = nc.dram_tensor("hs_loc", [SQ, H], bf16, kind="ExternalInput")
    rel = nc.dram_tensor("rel", [SQ, S, H], bf16, kind="ExternalInput")
    mask = nc.dram_tensor("mask", [S], f32, kind="ExternalInput")
    Wq = nc.dram_tensor("Wq", [H, H], bf16, kind="ExternalInput")
    Wk = nc.dram_tensor("Wk", [H, H], bf16, kind="ExternalInput")
    Wv = nc.dram_tensor("Wv", [H, H], bf16, kind="ExternalInput")
    Wr = nc.dram_tensor("Wr", [H, H], bf16, kind="ExternalInput")
    u_in = nc.dram_tensor("u", [H], f32, kind="ExternalInput")
    v_in = nc.dram_tensor("v", [H], f32, kind="ExternalInput")
    out = nc.dram_tensor("out", [SQ, H], f32, kind="ExternalOutput")

    with tile.TileContext(nc) as tc:
        with (
            tc.tile_pool(name="persist", bufs=1) as persist,
            tc.tile_pool(name="relbf", bufs=8) as relbf,
            tc.tile_pool(name="reltp", bufs=2) as reltp,
            tc.tile_pool(name="bdsb", bufs=2) as bdsb,
        ):
            ident_bf = persist.tile([P, P], bf16)
            make_identity(nc, ident_bf)
            ident_f32 = persist.tile([P, P], f32)
            make_identity(nc, ident_f32)
            ones_bf = persist.tile([P, 1], bf16)
            nc.vector.memset(ones_bf, 1.0)

            mask_sb = persist.tile([P, KT], f32)
            nc.gpsimd.dma_start(out=mask_sb, in_=mask.rearrange("(kt p) -> p kt", p=P))
            u_sb = persist.tile([P, FC], f32)
            nc.gpsimd.dma_start(out=u_sb, in_=u_in.rearrange("(c p) -> p c", p=P))
            v_sb = persist.tile([P, FC], f32)
            nc.gpsimd.dma_start(out=v_sb, in_=v_in.rearrange("(c p) -> p c", p=P))

            # ---- load hs / hs_loc / weights (host pre-cast to bf16) ----
            hs_bf = persist.tile([P, KT, H], bf16)       # [s-tile part, kt, i]
            nc.gpsimd.dma_start(out=hs_bf, in_=hs.rearrange("(kt p) i -> p kt i", p=P))

            hsl_bf = persist.tile([SQ, H], bf16)
            nc.gpsimd.dma_start(out=hsl_bf, in_=hs_loc[:, :])

            w_bf = {}
            for name, w in (("q", Wq), ("k", Wk), ("v", Wv), ("r", Wr)):
                wt = persist.tile([P, FC, H], bf16, name=f"w_{name}")  # [o-chunk part, oc, i]
                w_bf[name] = wt
                nc.gpsimd.dma_start(out=wt, in_=w.rearrange("(oc p) i -> p oc i", p=P))

            # ---- transpose hs and Wq/Wk/Wv (Wr stays natural) ----
            pproj_cm = tc.tile_pool(name="pproj", bufs=6, space="PSUM")
            pproj = pproj_cm.__enter__()
            psetup_cm = tc.tile_pool(name="psetup", bufs=2, space="PSUM")
            psetup = psetup_cm.__enter__()
            hsT = persist.tile([P, FC, S], bf16)          # [i part, ic, s]
            for ic in range(FC):
                for kt in range(KT):
                    pt = psetup.tile([P, P], bf16, tag="pt")
                    nc.tensor.transpose(pt, hs_bf[:, kt, ic * P:(ic + 1) * P], ident_bf)
                    nc.vector.tensor_copy(out=hsT[:, ic, kt * P:(kt + 1) * P], in_=pt)

            hslT = persist.tile([P, FC, SQ], bf16)        # [i part, ic, q]
            for ic in range(FC):
                pt = psetup.tile([P, SQ], bf16, tag="pt")
                nc.tensor.transpose(pt, hsl_bf[:, ic * P:(ic + 1) * P], ident_bf[:SQ, :SQ])
                nc.vector.tensor_copy(out=hslT[:, ic, :], in_=pt)

            wT = {}
            for name in ("q", "k", "v"):
                dst = persist.tile([P, FC, H], bf16, name=f"wT_{name}")  # [i part, ic, o]
                wT[name] = dst
                for ic in range(FC):
                    for oc in range(FC):
                        pt = psetup.tile([P, P], bf16, tag="pt")
                        nc.tensor.transpose(
                            pt, w_bf[name][:, oc, ic * P:(ic + 1) * P], ident_bf)
                        nc.vector.tensor_copy(
                            out=dst[:, ic, oc * P:(oc + 1) * P], in_=pt)

            # ---- projections (all bf16, fp32 PSUM accum) ----
            # kT[o, k] = sum_i Wk[o,i] hs[k,i] -> lhsT = WkT[i, o], rhs = hsT[i, k]
            kT_sb = persist.tile([P, FC, S], bf16)        # [o part, oc, k]
            if True:
                psetup_cm.__exit__(None, None, None)
                for oc in range(FC):
                    pp = pproj.tile([P, S], f32, tag="pp")

                    for ic in range(FC):
                        nc.tensor.matmul(
                            pp, wT["k"][:, ic, oc * P:(oc + 1) * P], hsT[:, ic, :],
                            start=(ic == 0), stop=(ic == FC - 1))
                    nc.vector.tensor_copy(out=kT_sb[:, oc, :], in_=pp)

                # quT/qvT[o, q] = q proj + u/v broadcast (over free dim)
                quT = persist.tile([P, FC, SQ], bf16)
                qvT = persist.tile([P, FC, SQ], bf16)
                for oc in range(FC):
                    pp = pproj.tile([P, SQ], f32, tag="pp")

                    for ic in range(FC):
                        nc.tensor.matmul(
                            pp, wT["q"][:, ic, oc * P:(oc + 1) * P], hslT[:, ic, :],
                            start=(ic == 0), stop=(ic == FC - 1))
                    nc.vector.tensor_scalar_add(
                        out=quT[:, oc, :], in0=pp, scalar1=u_sb[:, oc:oc + 1])
                    nc.vector.tensor_scalar_add(
                        out=qvT[:, oc, :], in0=pp, scalar1=v_sb[:, oc:oc + 1])

                # val[k, o] = sum_i hs[k,i] Wv[o,i] -> lhsT = hsT[i, k-tile], rhs = WvT[i, o]
                val_sb = persist.tile([P, KT, H], bf16)   # [k part, kt, o]
                for kt in range(KT):
                    for half in range(2):                 # N=384 chunks (<=512)
                        pp = pproj.tile([P, H // 2], f32, tag="pp")

                        for ic in range(FC):
                            nc.tensor.matmul(
                                pp, hsT[:, ic, kt * P:(kt + 1) * P],
                                wT["v"][:, ic, half * (H // 2):(half + 1) * (H // 2)],
                                start=(ic == 0), stop=(ic == FC - 1))
                        nc.vector.tensor_copy(
                            out=val_sb[:, kt, half * (H // 2):(half + 1) * (H // 2)],
                            in_=pp)

                # gT[F, (h,q)] = sum_J Wr[J, F] * qvBD[J, (h,q)]  (block-diag over heads)
                qvBD = persist.tile([P, FC, NH, SQ], bf16)
                nc.vector.memset(qvBD, 0.0)
                for h in range(NH):
                    jc, off = h // 2, (h % 2) * 64
                    nc.vector.tensor_copy(
                        out=qvBD[off:off + 64, jc, h, :], in_=qvT[off:off + 64, jc, :])

                pgac_cm = tc.tile_pool(name="pgac", bufs=2, space="PSUM")
                pgac = pgac_cm.__enter__()
                wr_dve = persist.tile([P, FC, H], bf16)
                nc.vector.tensor_copy(out=wr_dve, in_=w_bf["r"])
                gT = persist.tile([P, FC, NH, SQ], bf16)  # [F part, ft, h, q]
                for ft in range(FC):
                    for half in range(2):                 # N=288 chunks
                        pp = pgac.tile([P, NH * SQ // 2], f32, tag="pg2")

                        for jc in range(FC):
                            nc.tensor.matmul(
                                pp, wr_dve[:, jc, ft * P:(ft + 1) * P],
                                qvBD[:, jc, :, :].rearrange("p h q -> p (h q)")[
                                    :, half * 288:(half + 1) * 288],
                                start=(jc == 0), stop=(jc == FC - 1))
                        nc.vector.tensor_copy(
                            out=gT[:, ft, :, :].rearrange("p h q -> p (h q)")[
                                :, half * 288:(half + 1) * 288],
                            in_=pp)

                # a_cT[k, h, q] = sum_d k[h*64+d, k-part] * qu[h*64+d, q]
                scoresT = persist.tile([P, KT, NH, SQ], f32)
                for h in range(NH):
                    oc, off = h // 2, (h % 2) * 64
                    for kt in range(KT):
                        pp = pgac.tile([P, SQ], f32, tag="pg2")

                        nc.tensor.matmul(
                            pp, kT_sb[off:off + 64, oc, kt * P:(kt + 1) * P],
                            quT[off:off + 64, oc, :], start=True, stop=True)
                        nc.vector.tensor_copy(out=scoresT[:, kt, h, :], in_=pp)

            pgac_cm.__exit__(None, None, None)
            pproj_cm.__exit__(None, None, None)

            # ---- main rel stream: per q row ----
            with (
                tc.tile_pool(name="prt", bufs=4, space="PSUM") as prt,
                tc.tile_pool(name="pbd", bufs=2, space="PSUM") as pbd,
                tc.tile_pool(name="pbdt", bufs=2, space="PSUM") as pbdt,
            ):
                for q in range(SQ):
                    rbf = relbf.tile([P, KT, H], bf16, tag="rbf")
                    nc.gpsimd.dma_start(
                        out=rbf, in_=rel[q].rearrange("(kt p) F -> p kt F", p=P))

                    pbd_t = pbd.tile([NH, S], f32, tag="bd")
                    for fcx in range(FC):
                        ptile = prt.tile([P, S], bf16, tag="rt")
                        for kt in range(KT):
                            nc.tensor.transpose(
                                ptile[:, kt * P:(kt + 1) * P],
                                rbf[:, kt, fcx * P:(fcx + 1) * P], ident_bf)
                        rT = reltp.tile([P, S], bf16, tag="rT")
                        nc.vector.tensor_copy(out=rT, in_=ptile)
                        # b_d[h, k] += sum_F gT[F, h, q] * relT[F, k]
                        nc.tensor.matmul(
                            pbd_t, gT[:, fcx, :, q], rT,
                            start=(fcx == 0), stop=(fcx == FC - 1))

                    bd = bdsb.tile([NH, S], f32, tag="bdq")
                    nc.vector.tensor_copy(out=bd, in_=pbd_t)
                    # transpose b_d [12, 384] -> [k, 12] per k-tile, add into scoresT
                    pt2 = pbdt.tile([P, KT, NH], f32, tag="bdt")
                    for kt in range(KT):
                        nc.tensor.transpose(
                            pt2[:, kt, :], bd[:, kt * P:(kt + 1) * P],
                            ident_f32[:NH, :NH])
                    for kt in range(KT):
                        nc.vector.tensor_add(
                            out=scoresT[:, kt, :, q], in0=scoresT[:, kt, :, q],
                            in1=pt2[:, kt, :])

            # ---- softmax (k on partitions) + context ----
            expT = persist.tile([P, KT, NH, SQ], bf16)
            for kt in range(KT):
                nc.scalar.activation(
                    out=expT[:, kt, :, :].rearrange("p h q -> p (h q)"),
                    in_=scoresT[:, kt, :, :].rearrange("p h q -> p (h q)"),
                    func=EXP, scale=1.0 / np.sqrt(D).item(),
                    bias=mask_sb[:, kt:kt + 1])

            out_sb = persist.tile([SQ, H], f32)
            with (
                tc.tile_pool(name="pden", bufs=1, space="PSUM") as pden,
                tc.tile_pool(name="pctx", bufs=2, space="PSUM") as pctx,
            ):
                pd = pden.tile([SQ, NH], f32)
                for h in range(NH):
                    for kt in range(KT):
                        nc.tensor.matmul(
                            pd[:, h:h + 1], expT[:, kt, h, :], ones_bf,
                            start=(kt == 0), stop=(kt == KT - 1))
                den_r = persist.tile([SQ, NH], f32)
                nc.vector.reciprocal(out=den_r, in_=pd)

                for h in range(NH):
                    pc = pctx.tile([SQ, D], f32, tag="ctx")
                    for kt in range(KT):
                        nc.tensor.matmul(
                            pc, expT[:, kt, h, :], val_sb[:, kt, h * D:(h + 1) * D],
                            start=(kt == 0), stop=(kt == KT - 1))
                    nc.vector.tensor_scalar_mul(
                        out=out_sb[:, h * D:(h + 1) * D], in0=pc,
                        scalar1=den_r[:, h:h + 1])

            nc.gpsimd.dma_start(out=out[:, :], in_=out_sb)

    nc.compile()
    return nc


def make_in_maps(inputs):
    import ml_dtypes
    bf = ml_dtypes.bfloat16
    hidden_states = np.asarray(inputs["hidden_states"], dtype=np.float32)
    rel_bf = np.asarray(inputs["rel_embedding"], dtype=np.float32)[0].astype(bf)
    attention_mask = np.asarray(inputs["attention_mask"], dtype=np.float32)

    hs = hidden_states[0].astype(bf)
    common = {
        "hs": hs,
        "mask": attention_mask.reshape(S),
        "Wq": np.asarray(inputs["Wq"], np.float32).astype(bf),
        "Wk": np.asarray(inputs["Wk"], np.float32).astype(bf),
        "Wv": np.asarray(inputs["Wv"], np.float32).astype(bf),
        "Wr": np.asarray(inputs["Wr"], np.float32).astype(bf),
        "u": np.asarray(inputs["u"], np.float32).reshape(H),
        "v": np.asarray(inputs["v"], np.float32).reshape(H),
    }
    in_maps = []
    for c in range(NCORES):
        sl = slice(c * SQ, (c + 1) * SQ)
        in_maps.append({
            **common,
            "hs_loc": np.ascontiguousarray(hs[sl]),
            "rel": np.ascontiguousarray(rel_bf[sl]),
        })
    return in_maps


def kernel(**inputs):
    if "nc" not in _CACHED:
        _CACHED["nc"] = build_kernel()
    nc = _CACHED["nc"]
    in_maps = make_in_maps(inputs)

    from concourse.bass_utils import run_bass_kernel_spmd
    res = run_bass_kernel_spmd(nc, in_maps, list(range(NCORES)))
    parts = [res.results[c]["out"] for c in range(NCORES)]
    return np.concatenate(parts, axis=0)[None].astype(np.float32)

